# revision 5
# baseline (speedup 1.0000x reference)
"""GNN message passing (gather + segment_sum) on 8 trn2 NeuronCores.

Strategy: shard by dst range (12500 nodes/core). Per core:
  - edges sorted by (src-window, dst-tile); chunk counts per (window, tile)
    uniformized across cores so all cores run one SPMD program.
  - lift: dma_gather (4 SWDGE queues) fetches x[src] rows (128B payload,
    256B stride) from a windowed DRAM table (int16 idx per 32767-node window).
  - scatter: per 128-edge chunk, build one-hot S[k,j] = (dst_rel[k]==j) on
    DVE via is_equal against an iota constant, then PE matmul
    psum[tile] += S^T @ msgs accumulates the whole 12500x32 output in PSUM.
  - drain PSUM -> SBUF -> DRAM; host concatenates core outputs.
"""
import sys

sys.path.insert(0, "/opt/trn_rl_repo")
sys.path.insert(0, "/root/problem")

import numpy as np

N_NODES = 100000
N_EDGES = 1600000
D = 32
NCORES = 8
NPC = N_NODES // NCORES  # 12500 dst nodes per core
W = 4  # src windows
WN = 32767  # nodes per window (int16 idx limit; idx 32767 = zero row)
TROWS = 32768 * W  # gather table rows
NT = (NPC + 127) // 128  # 98 dst tiles per core
SLAB = 64  # gather columns per call (8192 idxs)
NQ = 4  # swdge queues

_cache = {}


def _build_and_compile(M_tw):
    import concourse.bass as bass
    import concourse.bacc as bacc
    import concourse.mybir as mybir
    import concourse.tile as tile

    M_cols = int(M_tw.sum())
    nc = bacc.Bacc("TRN2", target_bir_lowering=False, debug=False,
                   num_swdge_queues=NQ)
    tab = nc.dram_tensor("tab", [TROWS, 64], mybir.dt.float32,
                         kind="ExternalInput")
    idxt = nc.dram_tensor("idxt", [128, 8 * M_cols], mybir.dt.int16,
                          kind="ExternalInput")
    drt = nc.dram_tensor("drt", [128, M_cols], mybir.dt.float32,
                         kind="ExternalInput")
    scr = nc.dram_tensor("scr", [NT * 128, D], mybir.dt.float32,
                         kind="ExternalOutput")

    iota_d = nc.dram_tensor("iota", [128, 128], mybir.dt.float32,
                            kind="ExternalInput")

    # column -> (w, t) in (w, t, r) lex order
    col_w = []
    col_t = []
    for w in range(W):
        for t in range(NT):
            for _ in range(int(M_tw[w, t])):
                col_w.append(w)
                col_t.append(t)
    # start/stop flags per PSUM bank (start=True zeroes the whole 2KB bank)
    first_col = {}
    last_col = {}
    for c, t in enumerate(col_t):
        bk = t // 16
        if bk not in first_col:
            first_col[bk] = c
        last_col[bk] = c
    # window column ranges
    wstart = [0] * (W + 1)
    for w in range(W):
        wstart[w + 1] = wstart[w] + int(M_tw[w].sum())

    with tile.TileContext(nc) as tc:
        with (
            tc.tile_pool(name="sb", bufs=1) as pool1,
            tc.tile_pool(name="gb", bufs=3) as gpool,
            tc.tile_pool(name="ib", bufs=3) as ipool,
            tc.tile_pool(name="sp", bufs=4) as spool,
            tc.tile_pool(name="ps", bufs=1, space="PSUM") as psum_pool,
        ):
            iota_s = pool1.tile([128, 128], mybir.dt.float32)
            nc.sync.dma_start(iota_s[:], iota_d[:])
            dr_s = pool1.tile([128, M_cols], mybir.dt.float32)
            nc.sync.dma_start(dr_s[:], drt[:])

            banks = []
            for _bk in range(7):
                bt = psum_pool.tile([128, 512], mybir.dt.float32,
                                    space="PSUM", tag=f"bank{_bk}")
                banks.append(bt)

            qn = 0
            for w in range(W):
                c0w, c1w = wstart[w], wstart[w + 1]
                c = c0w
                while c < c1w:
                    ncols = min(SLAB, c1w - c)
                    ni = 128 * ncols
                    idx_s = ipool.tile([128, 8 * SLAB], mybir.dt.int16,
                                       tag="idx")
                    nc.sync.dma_start(idx_s[:, : 8 * ncols],
                                      idxt[:, 8 * c : 8 * (c + ncols)])
                    gbuf = gpool.tile([128, SLAB, D], mybir.dt.float32,
                                      tag="gb")
                    nc.gpsimd.dma_gather(
                        out_ap=gbuf[:, :ncols, :],
                        in_ap=tab[32768 * w : 32768 * (w + 1), :D],
                        idxs_ap=idx_s[:, : 8 * ncols],
                        num_idxs=ni,
                        num_idxs_reg=ni,
                        elem_size=D,
                        elem_step=64,
                        single_packet=False,
                        queue_num=qn,
                    )
                    qn = (qn + 1) % NQ
                    for j in range(ncols):
                        col = c + j
                        t = col_t[col]
                        s_t = spool.tile([128, 128], mybir.dt.float32,
                                         tag="sel")
                        nc.vector.tensor_tensor(
                            out=s_t[:],
                            in0=dr_s[:, col : col + 1].to_broadcast([128, 128]),
                            in1=iota_s[:],
                            op=mybir.AluOpType.is_equal,
                        )
                        bk = t // 16
                        off = (t % 16) * 32
                        nc.tensor.matmul(
                            out=banks[bk][:, off : off + 32],
                            lhsT=s_t[:],
                            rhs=gbuf[:, j, :],
                            start=(first_col[bk] == col),
                            stop=(last_col[bk] == col),
                            skip_group_check=True,
                        )
                    c += ncols

            stage = pool1.tile([128, NT * 32], mybir.dt.float32)
            for bk in range(7):
                n = 512 if bk < 6 else (NT * 32 - 3072)
                nc.vector.tensor_copy(stage[:, 512 * bk : 512 * bk + n],
                                      banks[bk][:, :n])
            nc.sync.dma_start(
                scr[:].rearrange("(t p) f -> p t f", p=128), stage[:]
            )

    nc.compile()
    return nc


def kernel(x, edge_index):
    from concourse.bass_utils import run_bass_kernel_spmd

    x = np.asarray(x, dtype=np.float32)
    ei = np.asarray(edge_index)
    src = ei[0].astype(np.int64)
    dst = ei[1].astype(np.int64)

    # gather table: row 32768*w + (v % WN) holds x[v]; row 32768*w+32767 = 0
    tab = np.zeros((TROWS, 64), dtype=np.float32)
    v = np.arange(N_NODES, dtype=np.int64)
    tab[32768 * (v // WN) + (v % WN), :D] = x

    core = dst // NPC
    w_all = src // WN
    iw_all = (src % WN).astype(np.int16)
    lt_all = (dst % NPC) // 128  # tile
    lr_all = (dst % NPC) % 128  # row within tile

    # counts per (core, w, t)
    key = (core * W + w_all) * NT + lt_all
    cnt = np.bincount(key, minlength=NCORES * W * NT).reshape(NCORES, W, NT)
    M_twc = -(-cnt // 128)  # ceil chunks
    M_tw = M_twc.max(axis=0)  # [W, NT] uniform across cores
    M_tw[0] = np.maximum(M_tw[0], 1)  # every tile written at least once

    ck = tuple(M_tw.ravel().tolist())
    if ck not in _cache:
        _cache[ck] = _build_and_compile(M_tw)
    nc = _cache[ck]

    M_cols = int(M_tw.sum())
    # column base per (w, t) in (w, t, r) lex order
    colbase = np.zeros((W, NT), dtype=np.int64)
    cb = np.cumsum(M_tw.ravel())
    colbase.ravel()[1:] = cb[:-1]

    # slab boundaries per window (for wrapped idx layout)
    wstart = np.zeros(W + 1, dtype=np.int64)
    wstart[1:] = np.cumsum(M_tw.sum(axis=1))

    in_maps = []
    for c in range(NCORES):
        m = core == c
        wv = w_all[m]
        iwv = iw_all[m]
        tv = lt_all[m]
        rv = lr_all[m].astype(np.float32)
        # position within (w, t) group
        gkey = wv * NT + tv
        order = np.argsort(gkey, kind="stable")
        gs = gkey[order]
        pos = np.arange(len(gs)) - np.repeat(
            np.concatenate(([0], np.cumsum(np.bincount(gs, minlength=W * NT))[:-1])),
            np.bincount(gs, minlength=W * NT),
        )
        col = colbase[wv[order], tv[order]] + pos // 128
        p = pos % 128

        idx_pc = np.full((128, M_cols), 32767, dtype=np.int16)
        dr_pc = np.full((128, M_cols), -1.0, dtype=np.float32)
        idx_pc[p, col] = iwv[order]
        dr_pc[p, col] = rv[order]

        # wrapped idx layout per gather call
        idx_dram = np.empty((128, 8 * M_cols), dtype=np.int16)
        for w in range(W):
            c0w, c1w = int(wstart[w]), int(wstart[w + 1])
            s = c0w
            while s < c1w:
                ncols = min(SLAB, c1w - s)
                ni = 128 * ncols
                vals = idx_pc[:, s : s + ncols].T.ravel()  # slot-major
                wrapped = np.tile(vals.reshape(ni // 16, 16).T, (8, 1))
                idx_dram[:, 8 * s : 8 * (s + ncols)] = wrapped
                s += ncols

        iota_np = np.tile(np.arange(128, dtype=np.float32)[None, :], (128, 1))
        in_maps.append({"tab": tab, "idxt": idx_dram, "drt": dr_pc,
                        "iota": iota_np})

    import os

    if os.environ.get("KERNEL_TRACE", "0") == "1":
        try:
            import axon_prof

            axon_prof.install()
        except Exception:
            pass
        res = run_bass_kernel_spmd(
            nc, in_maps, core_ids=list(range(NCORES)), trace=True, trace_cores=[0]
        )
        print(f"HW exec time: {res.exec_time_ns} ns")
        print("scopes:", res.per_core_scope_times)
    else:
        res = run_bass_kernel_spmd(nc, in_maps, core_ids=list(range(NCORES)))
    out = np.empty((N_NODES, D), dtype=np.float32)
    for c in range(NCORES):
        out[c * NPC : (c + 1) * NPC] = res.results[c]["scr"][:NPC]
    return out


# revision 6
# speedup vs baseline: 1.4669x; 1.4669x over previous
"""GNN message passing (gather + segment_sum) on 8 trn2 NeuronCores.

Strategy: shard by dst range (12500 nodes/core). Per core:
  - edges sorted by (src-window, dst-tile); chunk counts per (window, tile)
    uniformized across cores so all cores run one SPMD program.
  - lift: dma_gather (4 SWDGE queues) fetches x[src] rows (128B payload,
    256B stride) from a windowed DRAM table (int16 idx per 32767-node window).
  - scatter: per 128-edge chunk, build one-hot S[k,j] = (dst_rel[k]==j) on
    DVE via is_equal against an iota constant, then PE matmul
    psum[tile] += S^T @ msgs accumulates the whole 12500x32 output in PSUM.
  - drain PSUM -> SBUF -> DRAM; host concatenates core outputs.
"""
import sys

sys.path.insert(0, "/opt/trn_rl_repo")
sys.path.insert(0, "/root/problem")

import numpy as np

N_NODES = 100000
N_EDGES = 1600000
D = 32
NCORES = 8
NPC = N_NODES // NCORES  # 12500 dst nodes per core
W = 4  # src windows
WN = 32767  # nodes per window (int16 idx limit; idx 32767 = zero row)
TROWS = 32768 * W  # gather table rows
NT = (NPC + 127) // 128  # 98 dst tiles per core
SLAB = 64  # gather columns per call (8192 idxs)
NQ = 4  # swdge queues

_cache = {}


def _build_and_compile(M_tw):
    import concourse.bass as bass
    import concourse.bacc as bacc
    import concourse.mybir as mybir
    import concourse.tile as tile

    M_cols = int(M_tw.sum())
    nc = bacc.Bacc("TRN2", target_bir_lowering=False, debug=False,
                   num_swdge_queues=NQ)
    tab = nc.dram_tensor("tab", [TROWS, 64], mybir.dt.float32,
                         kind="ExternalInput")
    idxt = nc.dram_tensor("idxt", [128, 8 * M_cols], mybir.dt.int16,
                          kind="ExternalInput")
    drt = nc.dram_tensor("drt", [128, M_cols], mybir.dt.float32,
                         kind="ExternalInput")
    scr = nc.dram_tensor("scr", [NT * 128, D], mybir.dt.float32,
                         kind="ExternalOutput")

    iota_d = nc.dram_tensor("iota", [128, 128], mybir.dt.float32,
                            kind="ExternalInput")

    # column -> (w, t) in (w, t, r) lex order
    col_w = []
    col_t = []
    for w in range(W):
        for t in range(NT):
            for _ in range(int(M_tw[w, t])):
                col_w.append(w)
                col_t.append(t)
    # start/stop flags per PSUM bank (start=True zeroes the whole 2KB bank)
    first_col = {}
    last_col = {}
    for c, t in enumerate(col_t):
        bk = t // 16
        if bk not in first_col:
            first_col[bk] = c
        last_col[bk] = c
    # window column ranges
    wstart = [0] * (W + 1)
    for w in range(W):
        wstart[w + 1] = wstart[w] + int(M_tw[w].sum())

    with tile.TileContext(nc) as tc:
        with (
            tc.tile_pool(name="sb", bufs=1) as pool1,
            tc.tile_pool(name="gb", bufs=6) as gpool,
            tc.tile_pool(name="ib", bufs=6) as ipool,
            tc.tile_pool(name="sp", bufs=16) as spool,
            tc.tile_pool(name="ps", bufs=1, space="PSUM") as psum_pool,
        ):
            iota_s = pool1.tile([128, 128], mybir.dt.float32)
            nc.sync.dma_start(iota_s[:], iota_d[:])
            dr_s = pool1.tile([128, M_cols], mybir.dt.float32)
            nc.sync.dma_start(dr_s[:], drt[:])

            banks = []
            for _bk in range(7):
                bt = psum_pool.tile([128, 512], mybir.dt.float32,
                                    space="PSUM", tag=f"bank{_bk}")
                banks.append(bt)

            qn = 0
            for w in range(W):
                c0w, c1w = wstart[w], wstart[w + 1]
                c = c0w
                while c < c1w:
                    ncols = min(SLAB, c1w - c)
                    ni = 128 * ncols
                    idx_s = ipool.tile([128, 8 * SLAB], mybir.dt.int16,
                                       tag="idx")
                    nc.sync.dma_start(idx_s[:, : 8 * ncols],
                                      idxt[:, 8 * c : 8 * (c + ncols)])
                    gbuf = gpool.tile([128, SLAB, D], mybir.dt.float32,
                                      tag="gb")
                    nc.gpsimd.dma_gather(
                        out_ap=gbuf[:, :ncols, :],
                        in_ap=tab[32768 * w : 32768 * (w + 1), :D],
                        idxs_ap=idx_s[:, : 8 * ncols],
                        num_idxs=ni,
                        num_idxs_reg=ni,
                        elem_size=D,
                        elem_step=64,
                        single_packet=False,
                        queue_num=qn,
                    )
                    qn = (qn + 1) % NQ
                    for j in range(ncols):
                        col = c + j
                        t = col_t[col]
                        s_t = spool.tile([128, 128], mybir.dt.float32,
                                         tag="sel")
                        nc.vector.tensor_tensor(
                            out=s_t[:],
                            in0=dr_s[:, col : col + 1].to_broadcast([128, 128]),
                            in1=iota_s[:],
                            op=mybir.AluOpType.is_equal,
                        )
                        bk = t // 16
                        off = (t % 16) * 32
                        nc.tensor.matmul(
                            out=banks[bk][:, off : off + 32],
                            lhsT=s_t[:],
                            rhs=gbuf[:, j, :],
                            start=(first_col[bk] == col),
                            stop=(last_col[bk] == col),
                            skip_group_check=True,
                        )
                    c += ncols

            stage = pool1.tile([128, NT * 32], mybir.dt.float32)
            for bk in range(7):
                n = 512 if bk < 6 else (NT * 32 - 3072)
                nc.vector.tensor_copy(stage[:, 512 * bk : 512 * bk + n],
                                      banks[bk][:, :n])
            nc.sync.dma_start(
                scr[:].rearrange("(t p) f -> p t f", p=128), stage[:]
            )

    nc.compile()
    return nc


def kernel(x, edge_index):
    from concourse.bass_utils import run_bass_kernel_spmd

    x = np.asarray(x, dtype=np.float32)
    ei = np.asarray(edge_index)
    src = ei[0].astype(np.int64)
    dst = ei[1].astype(np.int64)

    # gather table: row 32768*w + (v % WN) holds x[v]; row 32768*w+32767 = 0
    tab = np.zeros((TROWS, 64), dtype=np.float32)
    v = np.arange(N_NODES, dtype=np.int64)
    tab[32768 * (v // WN) + (v % WN), :D] = x

    core = dst // NPC
    w_all = src // WN
    iw_all = (src % WN).astype(np.int16)
    lt_all = (dst % NPC) // 128  # tile
    lr_all = (dst % NPC) % 128  # row within tile

    # counts per (core, w, t)
    key = (core * W + w_all) * NT + lt_all
    cnt = np.bincount(key, minlength=NCORES * W * NT).reshape(NCORES, W, NT)
    M_twc = -(-cnt // 128)  # ceil chunks
    M_tw = M_twc.max(axis=0)  # [W, NT] uniform across cores
    M_tw[0] = np.maximum(M_tw[0], 1)  # every tile written at least once

    ck = tuple(M_tw.ravel().tolist())
    if ck not in _cache:
        _cache[ck] = _build_and_compile(M_tw)
    nc = _cache[ck]

    M_cols = int(M_tw.sum())
    # column base per (w, t) in (w, t, r) lex order
    colbase = np.zeros((W, NT), dtype=np.int64)
    cb = np.cumsum(M_tw.ravel())
    colbase.ravel()[1:] = cb[:-1]

    # slab boundaries per window (for wrapped idx layout)
    wstart = np.zeros(W + 1, dtype=np.int64)
    wstart[1:] = np.cumsum(M_tw.sum(axis=1))

    in_maps = []
    for c in range(NCORES):
        m = core == c
        wv = w_all[m]
        iwv = iw_all[m]
        tv = lt_all[m]
        rv = lr_all[m].astype(np.float32)
        # position within (w, t) group
        gkey = wv * NT + tv
        order = np.argsort(gkey, kind="stable")
        gs = gkey[order]
        pos = np.arange(len(gs)) - np.repeat(
            np.concatenate(([0], np.cumsum(np.bincount(gs, minlength=W * NT))[:-1])),
            np.bincount(gs, minlength=W * NT),
        )
        col = colbase[wv[order], tv[order]] + pos // 128
        p = pos % 128

        idx_pc = np.full((128, M_cols), 32767, dtype=np.int16)
        dr_pc = np.full((128, M_cols), -1.0, dtype=np.float32)
        idx_pc[p, col] = iwv[order]
        dr_pc[p, col] = rv[order]

        # wrapped idx layout per gather call
        idx_dram = np.empty((128, 8 * M_cols), dtype=np.int16)
        for w in range(W):
            c0w, c1w = int(wstart[w]), int(wstart[w + 1])
            s = c0w
            while s < c1w:
                ncols = min(SLAB, c1w - s)
                ni = 128 * ncols
                vals = idx_pc[:, s : s + ncols].T.ravel()  # slot-major
                wrapped = np.tile(vals.reshape(ni // 16, 16).T, (8, 1))
                idx_dram[:, 8 * s : 8 * (s + ncols)] = wrapped
                s += ncols

        iota_np = np.tile(np.arange(128, dtype=np.float32)[None, :], (128, 1))
        in_maps.append({"tab": tab, "idxt": idx_dram, "drt": dr_pc,
                        "iota": iota_np})

    import os

    if os.environ.get("KERNEL_TRACE", "0") == "1":
        try:
            import axon_prof

            axon_prof.install()
        except Exception:
            pass
        res = run_bass_kernel_spmd(
            nc, in_maps, core_ids=list(range(NCORES)), trace=True, trace_cores=[0]
        )
        print(f"HW exec time: {res.exec_time_ns} ns")
        print("scopes:", res.per_core_scope_times)
    else:
        res = run_bass_kernel_spmd(nc, in_maps, core_ids=list(range(NCORES)))
    out = np.empty((N_NODES, D), dtype=np.float32)
    for c in range(NCORES):
        out[c * NPC : (c + 1) * NPC] = res.results[c]["scr"][:NPC]
    return out
